# revision 1
# baseline (speedup 1.0000x reference)
"""Trainium2 Bass kernel for nn_FFTChainMatrix (block-circulant matmul via 64-pt rFFT).

y = x @ W.T where W is 4096x4096 block-circulant (64x64 grid of 64x64 circulant
blocks) built from channel-weighted circulant_params.  Computed in the FFT
domain as three 128x128-matmul stages per 512-token shard:

  T_in   PE-transpose x (tok-major) -> feature-major
  S1     rfft along block dim:      X1 = A_bd.T @ xt      (per 128-feat chunk)
  shuf   i-pair-major -> freq-pair-major: 32 big SBUF->SBUF DMAs
         X2[:, f*T:+T] <- X1[4f:4f+4, :]   (4-partition rows -> 128-part tile)
  S2     per-freq complex multiply+sum over blocks: Y2 = G[fp].T @ X2
  unshuf inverse: Y3[4f:4f+4, :] <- Y2[:, f*T:+T]
  S3     irfft:                     Y4 = B_bd.T @ Y3
  T_out  PE-transpose back to tok-major, DMA out

Sharding: data-parallel over tokens, 4096 tokens -> 8 cores x 512.
Matmul stages + transposes run as float32r (full-rate fp32 path on the PE).
"""

from contextlib import ExitStack

import numpy as np

BLK = 64
NB = 64           # blocks per side
T = 512           # tokens per core
NCORES = 8
FEAT = 4096

MM_DT = "f32r"    # "f32r" (fast) or "f32" (exact, 4x slower stages)


# ---------------------------------------------------------------- host math
def _build_matrices(circulant_params, channel_weights):
    """A1 (2,2,128,128), G (32,128,128), B1 (2,128,256), float32 (exact f64 math)."""
    c_w = np.einsum(
        "m,moid->oid",
        np.asarray(channel_weights, np.float64),
        np.asarray(circulant_params, np.float64),
    )
    Chat = np.fft.rfft(c_w, axis=-1)
    Wr, Wi = Chat.real, Chat.imag

    r = np.arange(BLK)
    A64 = np.zeros((BLK, BLK))
    A64[0, :] = 1.0
    A64[1, :] = (-1.0) ** r
    B64 = np.zeros((BLK, BLK))
    B64[:, 0] = 1.0 / BLK
    B64[:, 1] = ((-1.0) ** r) / BLK
    for p in range(1, 32):
        cc = np.cos(2 * np.pi * p * r / BLK)
        ss = np.sin(2 * np.pi * p * r / BLK)
        A64[2 * p, :] = cc
        A64[2 * p + 1, :] = -ss
        B64[:, 2 * p] = 2.0 * cc / BLK
        B64[:, 2 * p + 1] = -2.0 * ss / BLK

    A1 = np.zeros((4, 4, 128, 128))
    for kq in range(4):
        for mu in range(4):
            for b in range(8):
                for fl in range(8):
                    for c1 in range(2):
                        A1[kq, mu, b * 16: b * 16 + 16, fl * 16 + c1 * 8 + b] = \
                            A64[2 * (8 * mu + fl) + c1, 16 * kq: 16 * kq + 16]

    iperm = (np.arange(NB) % 8) * 8 + np.arange(NB) // 8
    G = np.zeros((32, 128, 128))
    for fp in range(32):
        if fp == 0:
            for i in range(NB):
                G[0, iperm[i], iperm] = Wr[:, i, 0]
                G[0, 64 + iperm[i], 64 + iperm] = Wr[:, i, 32]
        else:
            for i in range(NB):
                G[fp, iperm[i], iperm] = Wr[:, i, fp]
                G[fp, 64 + iperm[i], iperm] = -Wi[:, i, fp]
                G[fp, iperm[i], 64 + iperm] = Wi[:, i, fp]
                G[fp, 64 + iperm[i], 64 + iperm] = Wr[:, i, fp]

    B1 = np.zeros((4, 128, 512))
    for mu in range(4):
        for fl in range(8):
            for c1 in range(2):
                for b in range(8):
                    B1[mu, fl * 16 + c1 * 8 + b, b * 64: b * 64 + 64] = \
                        B64[:, 2 * (8 * mu + fl) + c1]

    return A1, G, B1


# ---------------------------------------------------------------- bass trace
def _trace_nc():
    import concourse.bass as bass
    import concourse.mybir as mybir
    import concourse.tile as tile
    from concourse import bacc
    from concourse.bass import ts

    f32 = mybir.dt.float32
    f16 = mybir.dt.float16
    TP = T

    nc = bacc.Bacc("TRN2", target_bir_lowering=False, debug=False,
                   num_devices=NCORES)
    x_h = nc.dram_tensor("x_shard", [FEAT, T], f16, kind="ExternalInput").ap()
    a_h = nc.dram_tensor("a1_mats", [128, 2048], f16, kind="ExternalInput").ap()
    g_h = nc.dram_tensor("g_mats", [128, 32 * 128], f16,
                         kind="ExternalInput").ap()
    b_h = nc.dram_tensor("b1_mats", [128, 2048], f16, kind="ExternalInput").ap()
    y_h = nc.dram_tensor("y_shard", [T, FEAT], f16, kind="ExternalOutput").ap()

    copy_ix = [0]
    ring_ix = [0]
    unshuf_ix = [0]

    with tile.TileContext(nc) as tc, ExitStack() as ctx:
        wpool = ctx.enter_context(tc.tile_pool(name="weights", bufs=1))
        ypool = ctx.enter_context(tc.tile_pool(name="yout", bufs=2))
        xtp = ctx.enter_context(tc.tile_pool(name="xtp", bufs=1))
        x2p = ctx.enter_context(tc.tile_pool(name="x2p", bufs=33))
        y2p = ctx.enter_context(tc.tile_pool(name="y2p", bufs=16))
        x1p = ctx.enter_context(tc.tile_pool(name="x1p", bufs=1))
        y3p = ctx.enter_context(tc.tile_pool(name="y3p", bufs=1))
        mm_ps = ctx.enter_context(tc.tile_pool(name="mm_ps", bufs=7, space="PSUM"))
        wu_ps = ctx.enter_context(tc.tile_pool(name="wu_ps", bufs=1, space="PSUM"))

        def copyback(out_ap, in_ap):
            if copy_ix[0] % 4 < 3:
                nc.vector.tensor_copy(out_ap, in_ap)
            else:
                nc.scalar.copy(out_ap, in_ap)
            copy_ix[0] += 1

        def shuf_dma(dst, srcap):
            eng = (nc.scalar, nc.sync, nc.gpsimd)[ring_ix[0] % 3]
            ring_ix[0] += 1
            return eng.dma_start(dst, srcap)

        def unshuf_dma(dst, srcap):
            eng = (nc.gpsimd, nc.sync, nc.scalar)[unshuf_ix[0] % 3]
            unshuf_ix[0] += 1
            return eng.dma_start(dst, srcap)

        a1t = wpool.tile([128, 2048], f16)
        nc.gpsimd.dma_start(a1t[:], a_h[:])
        gts = wpool.tile([128, 32 * 128], f16)
        nc.gpsimd.dma_start(gts[:], g_h[:])
        b1t = wpool.tile([128, 2048], f16)
        nc.gpsimd.dma_start(b1t[:], b_h[:])

        # ---- T_in: bulk load of pre-transposed x into group/half layout
        # xt_all[b*32+rh, (2g+kh)*T + t] = x_feat_major[(4g+b)*64+32kh+rh, t]
        xt_all = xtp.tile([128, 32 * TP], f16, tag="xt")
        dst_r = xt_all[:].rearrange("p (g four t) -> p g four t", four=4, t=TP)
        load_rr = [nc.sync, nc.scalar, nc.gpsimd]
        li = 0
        for kq in range(4):
            for b in range(8):
                # x row (8g+b)*64 + 16kq + rq, rq in [0,16)
                src_ap = bass.AP(
                    x_h.tensor, x_h.offset + (b * 64 + 16 * kq) * T,
                    [[T, 16], [512 * T, 8], [1, T]])
                load_rr[li % 3].dma_start(
                    dst_r[b * 16:(b + 1) * 16, :, kq, :], src_ap)
                li += 1

        # ---- S1 (2-way K accumulation per (g, mu))
        x1 = x1p.tile([128, 32 * TP], f16, tag="x1")
        for g in range(8):
            for mu in range(4):
                ps = mm_ps.tile([128, TP], f32, tag="mm")
                for kq in range(4):
                    nc.tensor.matmul(
                        ps[:], a1t[:, ts(4 * kq + mu, 128)],
                        xt_all[:, ts(4 * g + kq, TP)],
                        start=(kq == 0), stop=(kq == 3))
                copyback(x1[:, ts(4 * g + mu, TP)], ps[:])

        # ---- shuffle (one DMA per freq-pair, 8-part dense src)
        y3 = y3p.tile([128, 32 * TP], f16, tag="y3")
        x2cs = []
        for f in range(32):
            mu, fl = f // 8, f % 8
            x2c = x2p.tile([128, TP], f16, tag="x2")
            src_ap = x1[fl * 16: fl * 16 + 16, :].rearrange(
                "p (g four t) -> p g four t", four=4, t=TP)[:, :, mu, :]
            shuf_dma(x2c[:], src_ap)
            # PE warm-keeper: tiny matmul on the freshly landed chunk
            wps = wu_ps.tile([128, 64], f32, tag="wu")
            nc.tensor.matmul(wps[:], a1t[:, :128], x2c[:, :64],
                             start=True, stop=True)
            x2cs.append(x2c)

        # ---- S2 + unshuffle
        for f in range(32):
            mu, fl = f // 8, f % 8
            ps = mm_ps.tile([128, TP], f32, tag="mm")
            nc.tensor.matmul(ps[:], gts[:, ts(f, 128)], x2cs[f][:],
                             start=True, stop=True)
            y2c = y2p.tile([128, TP], f16, tag="y2")
            copyback(y2c[:], ps[:])
            dst_ap = y3[fl * 16: fl * 16 + 16, :].rearrange(
                "p (g four t) -> p g four t", four=4, t=TP)[:, :, mu, :]
            unshuf_dma(dst_ap, y2c[:])

        # ---- S3 fused with T_out (2-way K accumulation over mu)
        for tt in range(4):
            ys = ypool.tile([128, FEAT], f16, tag="yout")
            for g in range(8):
                ps = mm_ps.tile([128, TP], f32, tag="mm")
                for mu in range(4):
                    nc.tensor.matmul(
                        ps[:],
                        y3[:, (4 * g + mu) * TP + tt * 128:
                           (4 * g + mu) * TP + tt * 128 + 128],
                        b1t[:, ts(mu, 512)],
                        start=(mu == 0), stop=(mu == 3))
                copyback(ys[:, ts(g, 512)], ps[:])
            nc.sync.dma_start(y_h[ts(tt, 128), :], ys[:])

    nc.compile()
    return nc


_CACHE = {}


def make_in_maps(x, circulant_params, channel_weights):
    xf = np.ascontiguousarray(np.asarray(x, np.float32)).reshape(-1, FEAT)
    assert xf.shape[0] == NCORES * T, f"unexpected token count {xf.shape}"
    A1, G, B1 = _build_matrices(circulant_params, channel_weights)
    a1_kfm = np.zeros((128, 2048), np.float16)
    for kq in range(4):
        for mu in range(4):
            a1_kfm[:, (4 * kq + mu) * 128:(4 * kq + mu + 1) * 128] = \
                A1[kq, mu].astype(np.float16)
    g_kfm = np.ascontiguousarray(
        G.transpose(1, 0, 2).reshape(128, 32 * 128).astype(np.float16))
    b1_kfm = np.concatenate(
        [B1[0], B1[1], B1[2], B1[3]], axis=1).astype(np.float16)
    xf16 = xf.astype(np.float16)
    return [
        {
            "x_shard": np.ascontiguousarray(xf16[c * T:(c + 1) * T].T),
            "a1_mats": a1_kfm,
            "g_mats": g_kfm,
            "b1_mats": b1_kfm,
        }
        for c in range(NCORES)
    ]


def kernel(x, circulant_params, channel_weights):
    from concourse.bass_utils import run_bass_kernel_spmd

    x = np.ascontiguousarray(np.asarray(x, np.float32))
    orig_shape = x.shape

    if "nc" not in _CACHE:
        _CACHE["nc"] = _trace_nc()
    nc = _CACHE["nc"]

    in_maps = make_in_maps(x, circulant_params, channel_weights)
    res = run_bass_kernel_spmd(nc, in_maps, core_ids=list(range(NCORES)))
    y = np.concatenate([res.results[c]["y_shard"] for c in range(NCORES)], axis=0)
    return y.astype(np.float32).reshape(orig_shape)

